# revision 3
# baseline (speedup 1.0000x reference)
"""Trainium2 Bass kernel for nn_EulerMisorientation3D (v7).

Math per voxel (Bunge ZXZ, scale [2pi, pi, 2pi]):
    u = cos(2pi(x0+x2)), v = cos(2pi(x0-x2)), c = cos(pi*x1) (same for xh)
    P4 = (1+u*uh)(1+c)(1+ch) + (1+v*vh)(1-c)(1-ch)   [= 4(1+z), z=cos(theta)]
    theta = 2*atan((8-P4)*absrsqrt(P4*(8-P4) + eps))
    out = mean(theta^2)

v7 vs v6 (63.5us):
  - Tail: AbsRsqrt+Arctan(+Square-accum) on ACT replaces ln/ln/sigmoid/atan/
    square; table order trig -> abs_rsqrt -> trig (3 loads, same count).
  - Custom 1-uop DVE ops GPAIR_M/GPAIR_P ((1-+sb)(1-+sbh)) and UV1
    (1+u*uh) replace the stock ee/gg/uv chains: fewer DVE cycles and
    custom ops don't suffer the 4x DVE<->gpsimd SBUF contention.
  - DVE emission interleaves product chunks between tile A-phases.
  - gpsimd adds only tiles 0-2; DVE TT-adds t3/t4 (gpsimd serialization
    was the v6 trig-phase pacer).
"""

import math

import numpy as np

import concourse.bacc as bacc
import concourse.tile as tile
from concourse.tile_rust import add_dep_helper
from concourse import mybir
from concourse.bass_utils import run_bass_kernel_spmd

F32 = mybir.dt.float32
F16 = mybir.dt.float16
AF = mybir.ActivationFunctionType
OP = mybir.AluOpType

N_CORES = 8
NVOX = 128 * 128 * 128
PER = NVOX // N_CORES          # 262144
P = 128
COLS = PER // P                # 2048
SZ = [512, 512, 512, 384, 128]
OFF = [sum(SZ[:i]) for i in range(len(SZ))]
T = len(SZ)
NH = 2
HD = COLS // NH                # 1024
PI = math.pi

CH = [(0, 1024), (1024, 1920), (1920, 2048)]
NCH = len(CH)

GPS_ADD_TILES = {1, 2, 3}

def build_bass():
    nc = bacc.Bacc("TRN2", target_bir_lowering=False, debug=False,
                   num_devices=N_CORES)
    xs = nc.declare_dram_parameter("xs", [3, PER], F32, isOutput=False)
    xh = nc.declare_dram_parameter("xh", [3, PER], F32, isOutput=False)
    out = nc.declare_dram_parameter("o", [P, NCH], F32, isOutput=True)

    xs_v = xs[:].rearrange("c (p q) -> p c q", p=P)
    xh_v = xh[:].rearrange("c (p q) -> p c q", p=P)

    with tile.TileContext(nc) as tc:
        with (
            tc.tile_pool(name="io", bufs=1) as io,
            tc.tile_pool(name="wk", bufs=1) as wk,
            tc.tile_pool(name="big", bufs=1) as big,
        ):
            acc = big.tile([P, NCH], F32, tag="acc")
            x1b = big.tile([P, 2, COLS], F32, tag="x1b")
            sb = big.tile([P, 2, COLS], F16, tag="sb")
            su4 = big.tile([P, 4, COLS], F16, tag="su4")

            b_mpi2 = big.tile([P, 1], F32, tag="b_mpi2")
            nc.vector.memset(b_mpi2, -PI / 2)
            b_zero = big.tile([P, 1], F32, tag="b_zero")
            nc.vector.memset(b_zero, 0.0)
            b_eps = big.tile([P, 1], F32, tag="b_eps")
            nc.vector.memset(b_eps, 1e-5)
            b_one = big.tile([P, 1], F32, tag="b_one")
            nc.vector.memset(b_one, 1.0)

            in02s = []
            for j in range(T):
                in02s.append(io.tile([P, 2, 2, SZ[j]], F32,
                                     tag=f"in02_{j}", name=f"in02_{j}"))

            def pair_dma(j):
                sl = slice(OFF[j], OFF[j] + SZ[j])
                nc.sync.dma_start(out=in02s[j][:, 0, :, :],
                                  in_=xs_v[:, 0:3:2, sl])
                nc.sync.dma_start(out=in02s[j][:, 1, :, :],
                                  in_=xh_v[:, 0:3:2, sl])

            def x1_dma(h):
                hs = slice(h * HD, (h + 1) * HD)
                nc.sync.dma_start(out=x1b[:, 0, hs], in_=xs_v[:, 1, hs])
                nc.sync.dma_start(out=x1b[:, 1, hs], in_=xh_v[:, 1, hs])

            pair_dma(0)
            x1_dma(0)
            pair_dma(1)
            pair_dma(2)
            x1_dma(1)
            pair_dma(3)
            pair_dma(4)

            act_chain = []
            ggs = [None, None]
            uv1s = {}
            pqs = {}
            rs = {}
            ths = {}

            def sb_sins(h):
                hs = slice(h * HD, (h + 1) * HD)
                act_chain.append(nc.scalar.activation(
                    sb[:, :, hs], x1b[:, :, hs], AF.Sin,
                    bias=b_mpi2[:], scale=PI))

            def emit_adds(j):
                in02 = in02s[j]
                m4 = wk.tile([P, 4, SZ[j]], F32, tag=f"m4_{j}",
                             name=f"m4_{j}")
                if j in GPS_ADD_TILES:
                    # gpsimd s-rows only: its serial chain gated the wraps
                    nc.gpsimd.tensor_add(m4[:, 0:2, :], in02[:, :, 0, :],
                                         in02[:, :, 1, :])
                    nc.vector.tensor_tensor(m4[:, 2:4, :], in02[:, :, 1, :],
                                            in02[:, :, 0, :], OP.subtract)
                else:
                    nc.vector.tensor_tensor(m4[:, 0:2, :], in02[:, :, 0, :],
                                            in02[:, :, 1, :], OP.add)
                    nc.vector.tensor_tensor(m4[:, 2:4, :], in02[:, :, 1, :],
                                            in02[:, :, 0, :], OP.subtract)
                return m4

            def emit_wrap_sin(j, m4):
                ks = slice(OFF[j], OFF[j] + SZ[j])
                nc.vector.add_range_wrap(
                    m4[:, 0:2, :], m4[:, 0:2, :], -0.75, 0.5, 1.0)
                nc.vector.add_range_wrap(
                    m4[:, 2:4, :], m4[:, 2:4, :], 0.25, 0.5, 1.0)
                act_chain.append(nc.scalar.activation(
                    su4[:, :, ks], m4[:], AF.Sin, bias=b_zero[:],
                    scale=2 * PI))

            def emit_G(h):
                hs = slice(h * HD, (h + 1) * HD)
                ee = wk.tile([P, 2, 2, HD], F16, tag=f"ee_{h}",
                             name=f"ee_{h}")
                # ee on ACT (Identity is in every table set; ACT is half-idle)
                act_chain.append(nc.scalar.activation(
                    ee[:, 0, :, :], sb[:, :, hs], AF.Identity,
                    bias=b_one[:], scale=-1.0))
                act_chain.append(nc.scalar.activation(
                    ee[:, 1, :, :], sb[:, :, hs], AF.Identity,
                    bias=b_one[:], scale=1.0))
                gg = wk.tile([P, 2, HD], F16, tag=f"gg_{h}", name=f"gg_{h}")
                nc.vector.tensor_mul(gg[:], ee[:, :, 0, :], ee[:, :, 1, :])
                ggs[h] = gg

            def emit_uv1(ci):
                c0, c1 = CH[ci]
                w = c1 - c0
                cs = slice(c0, c1)
                uv = wk.tile([P, 2, w], F16, tag=f"uv_{ci}", name=f"uv_{ci}")
                nc.vector.tensor_mul(uv[:], su4[:, 0::2, cs],
                                     su4[:, 1::2, cs])
                nc.vector.tensor_scalar(uv[:], uv[:], 1.0, None, OP.add)
                uv1s[ci] = uv

            def emit_mmpq(ci):
                c0, c1 = CH[ci]
                w = c1 - c0
                h = 0 if c1 <= HD else 1
                go = c0 - h * HD
                mm = wk.tile([P, 2, w], F16, tag=f"mm_{ci}", name=f"mm_{ci}")
                nc.vector.tensor_mul(mm[:], uv1s[ci][:],
                                     ggs[h][:, :, go:go + w])
                pq = wk.tile([P, w], F16, tag=f"pq_{ci}", name=f"pq_{ci}")
                nc.vector.tensor_tensor(pq[:], mm[:, 0, :], mm[:, 1, :],
                                        OP.add)
                q8 = wk.tile([P, w], F16, tag=f"q8_{ci}", name=f"q8_{ci}")
                nc.vector.tensor_scalar(q8[:], pq[:], -1.0, 8.0,
                                        OP.mult, OP.add)
                mt = wk.tile([P, w], F16, tag=f"mt_{ci}", name=f"mt_{ci}")
                nc.vector.tensor_tensor(mt[:], q8[:], pq[:], OP.mult)
                pqs[ci] = (q8, mt)

            rsq_acts = []
            tail_acts = []

            def emit_rsq(ci):
                c0, c1 = CH[ci]
                w = c1 - c0
                _, mt = pqs[ci]
                r = wk.tile([P, w], F16, tag=f"r_{ci}", name=f"r_{ci}")
                rsq_acts.append(nc.scalar.activation(
                    r[:], mt[:], AF.Abs_reciprocal_sqrt, bias=b_eps[:]))
                rs[ci] = r

            def emit_y(ci):
                c0, c1 = CH[ci]
                w = c1 - c0
                q8, _ = pqs[ci]
                y = wk.tile([P, w], F16, tag=f"y_{ci}", name=f"y_{ci}")
                nc.vector.tensor_tensor(y[:], q8[:], rs[ci][:], OP.mult)
                ths[ci] = y

            def emit_atan_sq(ci):
                c0, c1 = CH[ci]
                w = c1 - c0
                th = wk.tile([P, w], F16, tag=f"th_{ci}", name=f"th_{ci}")
                tail_acts.append(nc.scalar.activation(
                    th[:], ths[ci][:], AF.Arctan))
                sq = wk.tile([P, w], F16, tag=f"sq_{ci}", name=f"sq_{ci}")
                nc.vector.scalar_tensor_tensor(
                    sq[:], th[:], 4.0, th[:], OP.mult, OP.mult,
                    accum_out=acc[:, ci:ci + 1])

            # ---- emission order = per-engine queue order ----
            m0 = emit_adds(0)      # DVE TT (fills DVE's early gap)
            m1 = emit_adds(1)      # gpsimd queue: t1, t2, t3
            m2 = emit_adds(2)
            emit_wrap_sin(0, m0)
            sb_sins(0)
            emit_wrap_sin(1, m1)
            emit_G(0)              # after sb-h0
            emit_wrap_sin(2, m2)
            sb_sins(1)             # x1h1 lands before t3; fill ACT gap
            m3 = emit_adds(3)
            emit_wrap_sin(3, m3)
            m4_ = emit_adds(4)     # DVE TT (tiny, t4 data ~26.6)
            emit_wrap_sin(4, m4_)
            emit_uv1(0)
            emit_mmpq(0)
            emit_G(1)
            emit_uv1(1)
            emit_mmpq(1)
            emit_uv1(2)
            emit_mmpq(2)
            emit_rsq(0)
            emit_rsq(1)
            emit_rsq(2)
            emit_y(0)
            emit_y(1)
            emit_y(2)
            emit_atan_sq(0)
            emit_atan_sq(1)
            emit_atan_sq(2)

            # ACT queue: trig (sins) -> absrsqrt set -> trig (atan+square).
            full_chain = act_chain + rsq_acts + tail_acts
            for a, b in zip(full_chain, full_chain[1:]):
                add_dep_helper(b.ins, a.ins, sync=False,
                               reason="ACT table-set ordering")

            nc.sync.dma_start(out=out[:], in_=acc[:])

    nc.compile()
    return nc


_CACHE = {}


def _get_nc():
    if "nc" not in _CACHE:
        _CACHE["nc"] = build_bass()
    return _CACHE["nc"]


def _run(x, x_hat, **spmd_kwargs):
    x = np.ascontiguousarray(np.asarray(x, dtype=np.float32).reshape(3, NVOX))
    xh = np.ascontiguousarray(np.asarray(x_hat, dtype=np.float32).reshape(3, NVOX))

    in_maps = []
    for c in range(N_CORES):
        sl = slice(c * PER, (c + 1) * PER)
        in_maps.append({
            "xs": np.ascontiguousarray(x[:, sl]),
            "xh": np.ascontiguousarray(xh[:, sl]),
        })

    nc = _get_nc()
    res = run_bass_kernel_spmd(
        nc, in_maps, core_ids=list(range(N_CORES)), **spmd_kwargs)
    total = 0.0
    for r in res.results:
        total += r["o"].astype(np.float64).sum()
    return np.float32(total / NVOX), res


def kernel(x: np.ndarray, x_hat: np.ndarray) -> np.ndarray:
    val, _ = _run(x, x_hat)
    return val


# revision 4
# speedup vs baseline: 1.0185x; 1.0185x over previous
"""Trainium2 Bass kernel for nn_EulerMisorientation3D (v7).

Math per voxel (Bunge ZXZ, scale [2pi, pi, 2pi]):
    u = cos(2pi(x0+x2)), v = cos(2pi(x0-x2)), c = cos(pi*x1) (same for xh)
    P4 = (1+u*uh)(1+c)(1+ch) + (1+v*vh)(1-c)(1-ch)   [= 4(1+z), z=cos(theta)]
    theta = 2*atan((8-P4)*absrsqrt(P4*(8-P4) + eps))
    out = mean(theta^2)

v7 vs v6 (63.5us):
  - Tail: AbsRsqrt+Arctan(+Square-accum) on ACT replaces ln/ln/sigmoid/atan/
    square; table order trig -> abs_rsqrt -> trig (3 loads, same count).
  - Custom 1-uop DVE ops GPAIR_M/GPAIR_P ((1-+sb)(1-+sbh)) and UV1
    (1+u*uh) replace the stock ee/gg/uv chains: fewer DVE cycles and
    custom ops don't suffer the 4x DVE<->gpsimd SBUF contention.
  - DVE emission interleaves product chunks between tile A-phases.
  - gpsimd adds only tiles 0-2; DVE TT-adds t3/t4 (gpsimd serialization
    was the v6 trig-phase pacer).
"""

import math

import numpy as np

import concourse.bacc as bacc
import concourse.tile as tile
from concourse.tile_rust import add_dep_helper
from concourse import mybir
from concourse.bass_utils import run_bass_kernel_spmd

F32 = mybir.dt.float32
F16 = mybir.dt.float16
AF = mybir.ActivationFunctionType
OP = mybir.AluOpType

N_CORES = 8
NVOX = 128 * 128 * 128
PER = NVOX // N_CORES          # 262144
P = 128
COLS = PER // P                # 2048
SZ = [512, 512, 512, 384, 128]
OFF = [sum(SZ[:i]) for i in range(len(SZ))]
T = len(SZ)
NH = 2
HD = COLS // NH                # 1024
PI = math.pi

CH = [(0, 1024), (1024, 1920), (1920, 2048)]
NCH = len(CH)

GPS_ADD_TILES = {1, 2, 3}

def build_bass():
    nc = bacc.Bacc("TRN2", target_bir_lowering=False, debug=False,
                   num_devices=N_CORES)
    xs = nc.declare_dram_parameter("xs", [3, PER], F32, isOutput=False)
    xh = nc.declare_dram_parameter("xh", [3, PER], F32, isOutput=False)
    out = nc.declare_dram_parameter("o", [P, NCH], F32, isOutput=True)

    xs_v = xs[:].rearrange("c (p q) -> p c q", p=P)
    xh_v = xh[:].rearrange("c (p q) -> p c q", p=P)

    with tile.TileContext(nc) as tc:
        with (
            tc.tile_pool(name="io", bufs=1) as io,
            tc.tile_pool(name="wk", bufs=1) as wk,
            tc.tile_pool(name="big", bufs=1) as big,
        ):
            acc = big.tile([P, NCH], F32, tag="acc")
            x1b = big.tile([P, 2, COLS], F32, tag="x1b")
            sb = big.tile([P, 2, COLS], F16, tag="sb")
            su4 = big.tile([P, 4, COLS], F16, tag="su4")

            b_mpi2 = big.tile([P, 1], F32, tag="b_mpi2")
            nc.vector.memset(b_mpi2, -PI / 2)
            b_zero = big.tile([P, 1], F32, tag="b_zero")
            nc.vector.memset(b_zero, 0.0)
            b_eps = big.tile([P, 1], F32, tag="b_eps")
            nc.vector.memset(b_eps, 1e-5)
            b_one = big.tile([P, 1], F32, tag="b_one")
            nc.vector.memset(b_one, 1.0)

            in02s = []
            for j in range(T):
                in02s.append(io.tile([P, 2, 2, SZ[j]], F32,
                                     tag=f"in02_{j}", name=f"in02_{j}"))

            def pair_dma(j):
                sl = slice(OFF[j], OFF[j] + SZ[j])
                nc.sync.dma_start(out=in02s[j][:, 0, :, :],
                                  in_=xs_v[:, 0:3:2, sl])
                nc.sync.dma_start(out=in02s[j][:, 1, :, :],
                                  in_=xh_v[:, 0:3:2, sl])

            def x1_dma(h):
                hs = slice(h * HD, (h + 1) * HD)
                nc.sync.dma_start(out=x1b[:, 0, hs], in_=xs_v[:, 1, hs])
                nc.sync.dma_start(out=x1b[:, 1, hs], in_=xh_v[:, 1, hs])

            pair_dma(0)
            x1_dma(0)
            pair_dma(1)
            pair_dma(2)
            x1_dma(1)
            pair_dma(3)
            pair_dma(4)

            act_chain = []
            ggs = {}
            uv1s = {}
            pqs = {}
            rs = {}
            ths = {}

            def sb_sins(h):
                hs = slice(h * HD, (h + 1) * HD)
                act_chain.append(nc.scalar.activation(
                    sb[:, :, hs], x1b[:, :, hs], AF.Sin,
                    bias=b_mpi2[:], scale=PI))

            def emit_adds(j):
                in02 = in02s[j]
                m4 = wk.tile([P, 4, SZ[j]], F32, tag=f"m4_{j}",
                             name=f"m4_{j}")
                if j in GPS_ADD_TILES:
                    # gpsimd s-rows only: its serial chain gated the wraps
                    nc.gpsimd.tensor_add(m4[:, 0:2, :], in02[:, :, 0, :],
                                         in02[:, :, 1, :])
                    nc.vector.tensor_tensor(m4[:, 2:4, :], in02[:, :, 1, :],
                                            in02[:, :, 0, :], OP.subtract)
                else:
                    nc.vector.tensor_tensor(m4[:, 0:2, :], in02[:, :, 0, :],
                                            in02[:, :, 1, :], OP.add)
                    nc.vector.tensor_tensor(m4[:, 2:4, :], in02[:, :, 1, :],
                                            in02[:, :, 0, :], OP.subtract)
                return m4

            def emit_wrap_sin(j, m4):
                ks = slice(OFF[j], OFF[j] + SZ[j])
                nc.vector.add_range_wrap(
                    m4[:, 0:2, :], m4[:, 0:2, :], -0.75, 0.5, 1.0)
                nc.vector.add_range_wrap(
                    m4[:, 2:4, :], m4[:, 2:4, :], 0.25, 0.5, 1.0)
                act_chain.append(nc.scalar.activation(
                    su4[:, :, ks], m4[:], AF.Sin, bias=b_zero[:],
                    scale=2 * PI))

            def emit_G(ci):
                # per-chunk ee/gg so tiny c2 clears the tail path early
                lo, hi = CH[ci]
                w = hi - lo
                hs = slice(lo, hi)
                ee = wk.tile([P, 2, 2, w], F16, tag=f"ee_{ci}",
                             name=f"ee_{ci}")
                # ee on ACT (Identity is in every table set; ACT is half-idle)
                act_chain.append(nc.scalar.activation(
                    ee[:, 0, :, :], sb[:, :, hs], AF.Identity,
                    bias=b_one[:], scale=-1.0))
                act_chain.append(nc.scalar.activation(
                    ee[:, 1, :, :], sb[:, :, hs], AF.Identity,
                    bias=b_one[:], scale=1.0))
                gg = wk.tile([P, 2, w], F16, tag=f"gg_{ci}", name=f"gg_{ci}")
                nc.vector.tensor_mul(gg[:], ee[:, :, 0, :], ee[:, :, 1, :])
                ggs[ci] = gg

            def emit_uv1(ci):
                c0, c1 = CH[ci]
                w = c1 - c0
                cs = slice(c0, c1)
                uv = wk.tile([P, 2, w], F16, tag=f"uv_{ci}", name=f"uv_{ci}")
                nc.vector.tensor_mul(uv[:], su4[:, 0::2, cs],
                                     su4[:, 1::2, cs])
                nc.vector.tensor_scalar(uv[:], uv[:], 1.0, None, OP.add)
                uv1s[ci] = uv

            def emit_mmpq(ci):
                c0, c1 = CH[ci]
                w = c1 - c0
                mm = wk.tile([P, 2, w], F16, tag=f"mm_{ci}", name=f"mm_{ci}")
                nc.vector.tensor_mul(mm[:], uv1s[ci][:], ggs[ci][:])
                pq = wk.tile([P, w], F16, tag=f"pq_{ci}", name=f"pq_{ci}")
                nc.vector.tensor_tensor(pq[:], mm[:, 0, :], mm[:, 1, :],
                                        OP.add)
                q8 = wk.tile([P, w], F16, tag=f"q8_{ci}", name=f"q8_{ci}")
                nc.vector.tensor_scalar(q8[:], pq[:], -1.0, 8.0,
                                        OP.mult, OP.add)
                mt = wk.tile([P, w], F16, tag=f"mt_{ci}", name=f"mt_{ci}")
                nc.vector.tensor_tensor(mt[:], q8[:], pq[:], OP.mult)
                pqs[ci] = (q8, mt)

            rsq_acts = []
            tail_acts = []

            def emit_rsq(ci):
                c0, c1 = CH[ci]
                w = c1 - c0
                _, mt = pqs[ci]
                r = wk.tile([P, w], F16, tag=f"r_{ci}", name=f"r_{ci}")
                rsq_acts.append(nc.scalar.activation(
                    r[:], mt[:], AF.Abs_reciprocal_sqrt, bias=b_eps[:]))
                rs[ci] = r

            def emit_y(ci):
                c0, c1 = CH[ci]
                w = c1 - c0
                q8, _ = pqs[ci]
                y = wk.tile([P, w], F16, tag=f"y_{ci}", name=f"y_{ci}")
                nc.vector.tensor_tensor(y[:], q8[:], rs[ci][:], OP.mult)
                ths[ci] = y

            def emit_atan_sq(ci):
                c0, c1 = CH[ci]
                w = c1 - c0
                th = wk.tile([P, w], F16, tag=f"th_{ci}", name=f"th_{ci}")
                tail_acts.append(nc.scalar.activation(
                    th[:], ths[ci][:], AF.Arctan))
                sq = wk.tile([P, w], F16, tag=f"sq_{ci}", name=f"sq_{ci}")
                nc.vector.scalar_tensor_tensor(
                    sq[:], th[:], 4.0, th[:], OP.mult, OP.mult,
                    accum_out=acc[:, ci:ci + 1])

            # ---- emission order = per-engine queue order ----
            m0 = emit_adds(0)      # DVE TT (fills DVE's early gap)
            m1 = emit_adds(1)      # gpsimd queue: t1, t2, t3
            m2 = emit_adds(2)
            emit_wrap_sin(0, m0)
            sb_sins(0)
            emit_wrap_sin(1, m1)
            emit_G(0)              # after sb-h0
            emit_wrap_sin(2, m2)
            sb_sins(1)             # x1h1 lands before t3; fill ACT gap
            emit_G(2)              # tiny c2 ee/gg: clears the tail path
            m3 = emit_adds(3)
            emit_wrap_sin(3, m3)
            m4_ = emit_adds(4)     # DVE TT (tiny, t4 data ~26.6)
            emit_wrap_sin(4, m4_)
            emit_uv1(0)
            emit_mmpq(0)
            emit_uv1(2)            # c2 products early (only needs sin-t4)
            emit_mmpq(2)
            emit_G(1)
            emit_uv1(1)
            emit_mmpq(1)
            emit_rsq(0)
            emit_rsq(2)
            emit_rsq(1)            # last in chain = latest-ready
            emit_y(0)
            emit_y(2)
            emit_y(1)
            emit_atan_sq(0)
            emit_atan_sq(1)
            emit_atan_sq(2)

            # ACT queue: trig (sins) -> absrsqrt set -> trig (atan+square).
            full_chain = act_chain + rsq_acts + tail_acts
            for a, b in zip(full_chain, full_chain[1:]):
                add_dep_helper(b.ins, a.ins, sync=False,
                               reason="ACT table-set ordering")

            nc.sync.dma_start(out=out[:], in_=acc[:])

    nc.compile()
    return nc


_CACHE = {}


def _get_nc():
    if "nc" not in _CACHE:
        _CACHE["nc"] = build_bass()
    return _CACHE["nc"]


def _run(x, x_hat, **spmd_kwargs):
    x = np.ascontiguousarray(np.asarray(x, dtype=np.float32).reshape(3, NVOX))
    xh = np.ascontiguousarray(np.asarray(x_hat, dtype=np.float32).reshape(3, NVOX))

    in_maps = []
    for c in range(N_CORES):
        sl = slice(c * PER, (c + 1) * PER)
        in_maps.append({
            "xs": np.ascontiguousarray(x[:, sl]),
            "xh": np.ascontiguousarray(xh[:, sl]),
        })

    nc = _get_nc()
    res = run_bass_kernel_spmd(
        nc, in_maps, core_ids=list(range(N_CORES)), **spmd_kwargs)
    total = 0.0
    for r in res.results:
        total += r["o"].astype(np.float64).sum()
    return np.float32(total / NVOX), res


def kernel(x: np.ndarray, x_hat: np.ndarray) -> np.ndarray:
    val, _ = _run(x, x_hat)
    return val
